# revision 1
# baseline (speedup 1.0000x reference)
"""ContextQueryAttention (BiDAF-style) Trainium2 kernel, v3.

Problem: nn_ContextQueryAttention_44066364457466
  query [B=8, Q=512, D=512], context [B=8, C=2048, D=512],
  query_weights/context_weights [D,1], dot_weights [D,D], mask all-True.
  out [B, C, 4D]: concat(context, c2q@query, context*that, context*qtc)

Sharding: data-parallel over batch. B == 8 == n_cores, one batch element
per NeuronCore, no collectives.

Math (per batch element; mask all-ones so it drops out):
  H[d,q]  = sum_e W[d,e] qT[e,q]       (contract the small side first:
  sim[c,q]= sum_d ctx[c,d] H[d,q] + cw[c] + qw[q]   saves 400M MACs vs
                                                    the (ctx@W)@qT order)
  E[c,q]  = exp(sim - m_c)   (m_c row max; no global sync needed)
  F       = u_c * E, u_c = exp(m_c - K), K global max => F = exp(sim - K)
  Both softmaxes come from F alone:
    c2q = F / rowsum(F)   (diag rescale cancels in row normalization)
    q2c = F / colsum(F)
  ctq = c2q @ query;  G[q,d] = q2c^T @ ctx;  qtc = c2q @ G
F stays in f32 range: the worst row max sits ~60-80 below the global max
and exp(-80) is still a normal f32; entries below exp(-87) flush to ~0
and contribute nothing to either softmax.

Implementation notes:
- All heavy matmuls run as float32r via bitcast (the PE rounds fp32 on
  ingest; pre-rounding copies are redundant). Transposes use the f32r
  identity (1.5 cyc/row).
- Four 128x128 transposes are packed per PSUM bank (start=True zeroes
  the 2KB region, siblings use start=False), drained by one 512-wide
  eviction; the ET evictions also produce the q2c column sums via the
  activation accumulator.
- cw/qw enter sim through one K=2 rank-1 matmul per c-tile.
- The context copy in the output is DMA'd straight from SBUF; ctq/qtc
  blocks are built from PSUM by independent engines (no cross-engine
  chains inside a tile).
- Emission interleaves ctx-chunk transposes, cw_row, and sim so the PE
  stream stays dense while later ctx chunks are still loading.
"""

import numpy as np

B, Q, C, D = 8, 512, 2048, 512
P = 128
QT, CT, DT = Q // P, C // P, D // P  # 4, 16, 4
N_CORES = 8

_NC_CACHE = {}


def ds(start, size):
    return slice(start, start + size)


def _emit_body(nc, tc, pools, aps):
    import concourse.mybir as mybir

    f32 = mybir.dt.float32
    f32r = mybir.dt.float32r
    Exp = mybir.ActivationFunctionType.Exp
    Copy = mybir.ActivationFunctionType.Copy
    Mult = mybir.AluOpType.mult
    Max = mybir.AluOpType.max
    Min = mybir.AluOpType.min
    Add = mybir.AluOpType.add
    AxX = mybir.AxisListType.X

    (constp, statp, ctxp, qfam, rot8, fpool, bigp, stagep, scrp,
     ps_mm, ps_tr, ps_st) = pools
    (q_r3, c_r3, w_r3, cw_r3, qw_r3, out_r3) = aps

    # ---- loads (small tensors first so the PE can start early) ----
    w_r = rot8.tile([P, DT, D], f32r, name="w_f", tag="r8")
    nc.sync.dma_start(w_r, w_r3)

    q_r = qfam.tile([P, QT, D], f32r, name="q_f", tag="q_f")
    nc.sync.dma_start(q_r, q_r3)

    cwqw_r = statp.tile([P, DT, 2], f32r, name="cwqw_f", tag="cwqw_f")
    nc.gpsimd.dma_start(cwqw_r[:, :, 0:1], cw_r3)
    nc.gpsimd.dma_start(cwqw_r[:, :, 1:2], qw_r3)
    cwqw_f = cwqw_r.bitcast(f32)

    ctx_r = ctxp.tile([P, CT, D], f32r, name="ctx_f", tag="ctx_f")
    for g in range(4):
        nc.sync.dma_start(ctx_r[:, ds(g * 4, 4), :], c_r3[:, ds(g * 4, 4), :])
    ctx_f = ctx_r.bitcast(f32)

    # ---- constants ----
    id_f = constp.tile([P, P], f32, name="id_f", tag="id_f")
    from concourse.masks import make_identity
    make_identity(nc, id_f)
    id_r = constp.tile([P, P], f32r, name="id_r", tag="id_r")
    nc.vector.tensor_copy(id_r, id_f)
    ones2_f = constp.tile([1, 2], f32, name="ones2_f", tag="ones2_f")
    nc.vector.memset(ones2_f, 1.0)
    ones_row_f = constp.tile([1, 512], f32, name="ones_row_f", tag="ones_row_f")
    nc.vector.memset(ones_row_f, 1.0)
    ones_row_r = constp.tile([1, 512], f32r, name="ones_row", tag="ones_row")
    nc.vector.tensor_copy(ones_row_r, ones_row_f)

    def packed_transpose(dst_ap, srcs, evict_engine, accum_out=None):
        """len(srcs) transposes into one PSUM bank, one wide eviction."""
        n = len(srcs)
        ps = ps_tr.tile([P, 512], f32r, name="ptr", tag="tr")
        for j, src in enumerate(srcs):
            nc.tensor.matmul(ps[:, ds(j * P, P)], src, id_r,
                             is_transpose=True, start=(j == 0),
                             stop=(j == n - 1))
        if accum_out is not None:
            nc.scalar.activation(dst_ap, ps[:, 0: n * P], Copy,
                                 accum_out=accum_out)
        elif evict_engine == "act":
            nc.scalar.copy(dst_ap, ps[:, 0: n * P])
        else:
            nc.vector.tensor_copy(dst_ap, ps[:, 0: n * P])

    # ---- wT [e,d] and qT [e,q] (w arrives first) ----
    wT = rot8.tile([P, DT, D], f32r, name="wT", tag="r8")
    for eb in range(DT):
        packed_transpose(
            wT[:, eb, :],
            [w_r[:, dt, ds(eb * P, P)] for dt in range(DT)], "dve")
    qT = rot8.tile([P, DT, Q], f32r, name="qT", tag="r8")
    for eb in range(DT):
        packed_transpose(
            qT[:, eb, :],
            [q_r[:, qt, ds(eb * P, P)] for qt in range(QT)], "dve")

    # ---- H[d,q] = sum_e wT[e,d] qT[e,q]  (the small D x Q product) ----
    H = rot8.tile([P, DT, Q], f32r, name="H", tag="r8")
    for dtile in range(DT):
        pm = ps_mm.tile([P, 512], f32, name="pm", tag="mm")
        for eb in range(DT):
            nc.tensor.matmul(pm, wT[:, eb, ds(dtile * P, P)], qT[:, eb, :],
                             start=(eb == 0), stop=(eb == DT - 1))
        # H' = H + cw_w[d]: folds the cw[c] similarity term into the
        # contraction (sum_d ctx[c,d] cw_w[d] = cw[c]) at zero extra cost
        nc.scalar.activation(H[:, dtile, :], pm,
                             mybir.ActivationFunctionType.Identity,
                             bias=cwqw_f[:, dtile, 0:1], scale=1.0)

    # ---- qw_row [1, Q] ----
    qw_row = statp.tile([1, Q], f32r, name="qw_row", tag="qw_row")
    pqw = ps_st.tile([1, Q], f32, name="pst", tag="st")
    for dt in range(DT):
        nc.tensor.matmul(pqw, cwqw_r[:, dt, 1:2], qT[:, dt, :],
                         start=(dt == 0), stop=(dt == DT - 1))
    nc.vector.tensor_copy(qw_row, pqw)

    # ---- per ctx chunk: cT transposes, cw_row, then sim/E for its cts ----
    cT = bigp.tile([P, DT, C], f32r, name="cT", tag="big")
    F_r = fpool.tile([P, CT, Q], f32r, name="F_t", tag="F_t")
    F_t = F_r
    negm = statp.tile([P, CT], f32, name="negm", tag="negm")
    s_col = statp.tile([P, CT], f32, name="s_col", tag="s_col")

    def emit_cT_chunk(g):
        for dt in range(DT):
            packed_transpose(
                cT[:, dt, ds(g * 512, 512)],
                [ctx_r[:, 4 * g + j, ds(dt * P, P)] for j in range(4)], "dve")

    def emit_sim_chunk(g):
        for ct in range(4 * g, 4 * g + 4):
            # the exact-context output block goes straight from SBUF now,
            # while the DMA queues are otherwise idle
            nc.sync.dma_start(out_r3[:, ct, 0:D], ctx_f[:, ct, :])
            pm = ps_mm.tile([P, 512], f32, name="pm", tag="mm")
            for dt in range(DT):
                nc.tensor.matmul(pm, cT[:, dt, ds(ct * P, P)], H[:, dt, :],
                                 start=(dt == 0), stop=False)
            nc.tensor.matmul(pm, ones_row_r[0:1, 0:P], qw_row[0:1, :],
                             start=False, stop=True)
            nc.vector.tensor_reduce(negm[:, ds(ct, 1)], pm, axis=AxX,
                                    op=Max, negate=True)
            nc.scalar.activation(F_t[:, ct, :], pm, Exp,
                                 bias=negm[:, ds(ct, 1)], scale=1.0,
                                 accum_out=s_col[:, ds(ct, 1)])

    # transposes of chunk g+1 are emitted before sim of chunk g so the PE
    # never catches up with the ctx input DMA
    emit_cT_chunk(0)
    emit_cT_chunk(1)
    emit_sim_chunk(0)
    emit_cT_chunk(2)
    emit_sim_chunk(1)
    emit_cT_chunk(3)
    emit_sim_chunk(2)
    emit_sim_chunk(3)

    # ---- ET [q,c] from unscaled E: the whole c2q side (ctq and the qtc
    # combine) is normalized by 1/s_col, so it never sees the tiny
    # global-max-rescaled values and is immune to u underflow ----
    ET = bigp.tile([P, QT, C], f32r, name="ET", tag="big")
    for g in range(4):
        for qt in range(QT):
            packed_transpose(
                ET[:, qt, ds(g * 512, 512)],
                [F_r[:, 4 * g + j, ds(qt * P, P)] for j in range(4)], "act")
    r_col = statp.tile([P, CT], f32, name="r_col", tag="r_col")
    nc.vector.reciprocal(r_col, s_col)

    # ---- global max K; u_c = exp(m_c - K) ----
    nkp = statp.tile([P, 1], f32, name="nkp", tag="nkp")
    nc.vector.tensor_reduce(nkp, negm, axis=AxX, op=Min)  # -max_q per row
    pkt = ps_st.tile([1, P], f32, name="pst", tag="st")
    nc.tensor.transpose(pkt, nkp, id_f)
    negK11 = statp.tile([1, 1], f32, name="negK11", tag="negK11")
    nc.vector.tensor_reduce(negK11, pkt, axis=AxX, op=Min)  # -K
    krow = statp.tile([1, P], f32, name="krow", tag="krow")
    nc.vector.tensor_copy(krow, negK11.to_broadcast([1, P]))
    pkb = ps_st.tile([P, 2], f32, name="pst", tag="st")
    nc.tensor.matmul(pkb, krow, ones2_f, start=True, stop=True)
    negK_col = statp.tile([P, 1], f32, name="negK_col", tag="negK_col")
    nc.vector.tensor_copy(negK_col, pkb[:, 0:1])

    m_col = statp.tile([P, CT], f32, name="m_col", tag="m_col")
    nc.vector.tensor_scalar_mul(m_col, negm, -1.0)
    u_r = statp.tile([P, CT], f32r, name="u_col", tag="u_col")
    # scale=1.0 only: negative activation scale miscomputes on HW
    nc.scalar.activation(u_r, m_col, Exp, bias=negK_col, scale=1.0)
    u_col = u_r.bitcast(f32)
    # rows with m_c - K below the f32 denormal range flush u to 0; their
    # true q2c column weight is < e^-87 / S_q, i.e. negligible, so the
    # flush is harmless on the q2c side and the c2q side never uses u.

    # ---- S_q[q] = sum_c u_c E[c,q] via rank-1 matmuls ----
    psq = ps_st.tile([1, Q], f32, name="pst", tag="st")
    for ct in range(CT):
        nc.tensor.matmul(psq, u_r[:, ds(ct, 1)], F_r[:, ct, :],
                         start=(ct == 0), stop=(ct == CT - 1))
    sq_row = statp.tile([1, Q], f32, name="sq_row", tag="sq_row")
    nc.vector.tensor_copy(sq_row, psq)
    psqt = ps_st.tile([P, QT], f32, name="pst", tag="st")
    for qt in range(QT):
        nc.tensor.matmul(psqt[:, ds(qt, 1)], sq_row[0:1, ds(qt * P, P)],
                         id_f[0:1, 0:1], is_transpose=True,
                         start=(qt == 0), stop=(qt == QT - 1))
    sq = statp.tile([P, QT], f32, name="sq", tag="sq")
    nc.vector.tensor_copy(sq, psqt)
    rq = statp.tile([P, QT], f32, name="rq", tag="rq")
    nc.vector.reciprocal(rq, sq)

    # ---- F = u*E (out-of-place + copy-back; in-place corrupts on HW) ----
    for ct in range(CT):
        sc = scrp.tile([P, Q], f32r, name="sc", tag="scr")
        if ct % 2 == 0:
            nc.scalar.mul(sc, F_t[:, ct, :], u_col[:, ds(ct, 1)])
        else:
            nc.vector.tensor_scalar_mul(sc, F_t[:, ct, :],
                                        u_col[:, ds(ct, 1)])
        nc.gpsimd.tensor_copy(F_t[:, ct, :], sc)

    # ---- G[q,d] = (1/S_q) sum_c F[c,q] ctx[c,d] ----
    G_r = rot8.tile([P, QT, D], f32r, name="G_f", tag="r8")
    G_f = G_r
    for qt in range(QT):
        pm = ps_mm.tile([P, 512], f32, name="pm", tag="mm")
        for ct in range(CT):
            nc.tensor.matmul(pm, F_r[:, ct, ds(qt * P, P)], ctx_r[:, ct, :],
                             start=(ct == 0), stop=(ct == CT - 1))
        nc.scalar.mul(G_f[:, qt, :], pm, rq[:, ds(qt, 1)])

    # ---- outputs per ct ----
    for ct in range(CT):
        pc = ps_mm.tile([P, 512], f32, name="pm", tag="mm")
        for qt in range(QT):
            nc.tensor.matmul(pc, ET[:, qt, ds(ct * P, P)], q_r[:, qt, :],
                             start=(qt == 0), stop=(qt == QT - 1))
        pq = ps_mm.tile([P, 512], f32, name="pm", tag="mm")
        for qt in range(QT):
            nc.tensor.matmul(pq, ET[:, qt, ds(ct * P, P)], G_r[:, qt, :],
                             start=(qt == 0), stop=(qt == QT - 1))
        st = stagep.tile([P, 3 * D], f32, name="st", tag="stage")
        # three writers on independent engines, no intra-tile chains,
        # everything built straight from PSUM
        nc.scalar.mul(st[:, 0:D], pc, r_col[:, ds(ct, 1)])
        nc.vector.scalar_tensor_tensor(st[:, ds(D, D)], pc,
                                       r_col[:, ds(ct, 1)], ctx_f[:, ct, :],
                                       op0=Mult, op1=Mult)
        nc.vector.scalar_tensor_tensor(st[:, ds(2 * D, D)], pq,
                                       r_col[:, ds(ct, 1)], ctx_f[:, ct, :],
                                       op0=Mult, op1=Mult)
        dma_eng = nc.sync if ct % 2 == 0 else nc.gpsimd
        dma_eng.dma_start(out_r3[:, ct, ds(D, 3 * D)], st)


def _build_bass(loop_n=1):
    import concourse.bass as bass  # noqa: F401
    import concourse.mybir as mybir
    import concourse.tile as tile
    from concourse import bacc

    f32 = mybir.dt.float32

    f32r = mybir.dt.float32r
    nc = bacc.Bacc("TRN2", debug=False, num_devices=N_CORES)
    q_d = nc.dram_tensor("query", [Q, D], f32r, kind="ExternalInput")
    c_d = nc.dram_tensor("context", [C, D], f32r, kind="ExternalInput")
    qw_d = nc.dram_tensor("query_weights", [D, 1], f32r, kind="ExternalInput")
    cw_d = nc.dram_tensor("context_weights", [D, 1], f32r, kind="ExternalInput")
    w_d = nc.dram_tensor("dot_weights", [D, D], f32r, kind="ExternalInput")
    out_d = nc.dram_tensor("out", [C, 4 * D], f32, kind="ExternalOutput")

    aps = (
        q_d.ap().rearrange("(t p) d -> p t d", p=P),
        c_d.ap().rearrange("(t p) d -> p t d", p=P),
        w_d.ap().rearrange("(t p) e -> p t e", p=P),
        cw_d.ap().rearrange("(t p) o -> p t o", p=P),
        qw_d.ap().rearrange("(t p) o -> p t o", p=P),
        out_d.ap().rearrange("(t p) f -> p t f", p=P),
    )

    with tile.TileContext(nc) as tc:
        with (
            tc.tile_pool(name="const", bufs=1) as constp,
            tc.tile_pool(name="stats", bufs=1) as statp,
            tc.tile_pool(name="ctxp", bufs=1) as ctxp,
            tc.tile_pool(name="qfam", bufs=1) as qfam,
            tc.tile_pool(name="rot8", bufs=2) as rot8,
            tc.tile_pool(name="fpool", bufs=1) as fpool,
            tc.tile_pool(name="big", bufs=2) as bigp,
            tc.tile_pool(name="stage", bufs=3) as stagep,
            tc.tile_pool(name="scr", bufs=3) as scrp,
            tc.tile_pool(name="ps_mm", bufs=4, space="PSUM") as ps_mm,
            tc.tile_pool(name="ps_tr", bufs=3, space="PSUM") as ps_tr,
            tc.tile_pool(name="ps_st", bufs=1, space="PSUM") as ps_st,
        ):
            pools = (constp, statp, ctxp, qfam, rot8, fpool, bigp, stagep,
                     scrp, ps_mm, ps_tr, ps_st)
            if loop_n > 1:
                # unroll several bodies per loop iteration: the For_i
                # all-engine barrier is expensive on this runtime, so
                # amortize it while keeping exactly loop_n body runs
                k = 8 if loop_n % 8 == 0 else (4 if loop_n % 4 == 0 else 1)
                with tc.For_i(0, loop_n // k, 1):
                    for _ in range(k):
                        _emit_body(nc, tc, pools, aps)
            else:
                _emit_body(nc, tc, pools, aps)
    nc.compile()
    return nc


def get_nc(loop_n=1):
    if loop_n not in _NC_CACHE:
        _NC_CACHE[loop_n] = _build_bass(loop_n)
    return _NC_CACHE[loop_n]


def kernel(query, context, query_weights, context_weights, dot_weights,
           mask=None):
    from concourse.bass_utils import run_bass_kernel_spmd

    query = np.ascontiguousarray(np.asarray(query, dtype=np.float32))
    context = np.ascontiguousarray(np.asarray(context, dtype=np.float32))
    query_weights = np.ascontiguousarray(np.asarray(query_weights, dtype=np.float32))
    context_weights = np.ascontiguousarray(np.asarray(context_weights, dtype=np.float32))
    dot_weights = np.ascontiguousarray(np.asarray(dot_weights, dtype=np.float32))
    # mask is all-True per the problem spec; NEG_INF * (~mask) == 0, so it
    # drops out of the computation entirely.

    nc = get_nc()
    in_maps = [
        {
            "query": query[b],
            "context": context[b],
            "query_weights": query_weights,
            "context_weights": context_weights,
            "dot_weights": dot_weights,
        }
        for b in range(B)
    ]
    res = run_bass_kernel_spmd(nc, in_maps, core_ids=list(range(N_CORES)))
    out = np.stack([res.results[b]["out"] for b in range(B)], axis=0)
    return np.ascontiguousarray(out.astype(np.float32))


if __name__ == "__main__":
    rng = np.random.default_rng(0)
    inputs = {
        "query": rng.standard_normal((B, Q, D), dtype=np.float32),
        "context": rng.standard_normal((B, C, D), dtype=np.float32),
        "query_weights": rng.standard_normal((D, 1), dtype=np.float32) * 0.05,
        "context_weights": rng.standard_normal((D, 1), dtype=np.float32) * 0.05,
        "dot_weights": rng.standard_normal((D, D), dtype=np.float32) * 0.05,
        "mask": np.ones((B, C, Q), dtype=bool),
    }
    out = kernel(**inputs)
    print("out", out.shape, out.dtype)



# revision 4
# speedup vs baseline: 11.8167x; 11.8167x over previous
"""ContextQueryAttention (BiDAF-style) Trainium2 kernel, v4.

Problem: nn_ContextQueryAttention_44066364457466
  query [B=8, Q=512, D=512], context [B=8, C=2048, D=512],
  query_weights/context_weights [D,1], dot_weights [D,D], mask all-True.
  out [B, C, 4D]: concat(context, c2q@query, context*that, context*qtc)

Sharding: data-parallel over batch. B == 8 == n_cores, one batch element
per NeuronCore, no collectives.

Math (per batch element; mask all-ones so it drops out):
  H[d,q]  = sum_e W[d,e] qT[e,q]       (contract the small side first:
  sim[c,q]= sum_d ctx[c,d] H[d,q] + cw[c] + qw[q]   saves 400M MACs vs
                                                    the (ctx@W)@qT order)
  Two-pass softmax around one global shift K = max sim:
    pass 1: evict raw sim to SBUF, row maxes m_c on the fly
    K = max_c m_c  (tiny)
    pass 2: F = exp(sim - K), row sums s_col via the act accumulator
  Both softmaxes come from this single F:
    c2q = F / rowsum(F)      (the -K shift cancels per row)
    q2c = F / colsum(F)      (colsum(F) = S_q exactly, no u rescale)
  ctq = c2q @ query;  G[q,d] = q2c^T @ ctx;  qtc = c2q @ G
F stays in f32 range: the worst row max sits ~60-80 below the global max
and exp(-80) is still a normal f32; entries below exp(-87) flush to ~0
and contribute nothing to either softmax.

v4 vs v3: the v3 scheme exp'd per-row (F = exp(sim - m_c)) and then
needed u_c = exp(m_c - K) rescales of all of F (16 out-of-place muls +
16 gpsimd copy-backs, ~30us of gpsimd busy that stalled the PE) plus 16
rank-1 matmuls for S_q and a [1,Q]->[P,QT] transpose dance. Exp'ing
against the global K directly kills all of that: S_q falls out of the
ET-transpose evictions via the activation accumulator, already in the
[q-part, QT] layout that G's eviction scale wants.

Implementation notes:
- All heavy matmuls run as float32r via bitcast (the PE rounds fp32 on
  ingest; pre-rounding copies are redundant). Transposes use the f32r
  identity (1.5 cyc/row).
- Four 128x128 transposes are packed per PSUM bank (start=True zeroes
  the 2KB region, siblings use start=False), drained by one 512-wide
  eviction; the ET evictions also produce the q2c column sums S_q via
  the activation accumulator.
- cw enters sim through the H eviction bias; qw through one K=1 rank-1
  matmul per c-tile.
- DMA queues: inputs + half the staged output blocks on sync (HWDGE),
  the context output copies + other half of staged blocks on gpsimd
  (free of compute work in v4), so input loads of body i+1 are not
  stuck behind body i's output drains on one queue.
- Emission interleaves ctx-chunk transposes, and sim so the PE
  stream stays dense while later ctx chunks are still loading.
"""

import numpy as np

B, Q, C, D = 8, 512, 2048, 512
P = 128
QT, CT, DT = Q // P, C // P, D // P  # 4, 16, 4
N_CORES = 8

_NC_CACHE = {}


def ds(start, size):
    return slice(start, start + size)


def _emit_body(nc, tc, pools, aps):
    import concourse.mybir as mybir

    f32 = mybir.dt.float32
    f32r = mybir.dt.float32r
    Exp = mybir.ActivationFunctionType.Exp
    Copy = mybir.ActivationFunctionType.Copy
    Mult = mybir.AluOpType.mult
    Max = mybir.AluOpType.max
    Min = mybir.AluOpType.min
    AxX = mybir.AxisListType.X

    (constp, statp, ctxp, qfam, rot8, fpool, simp, bigp, stagep,
     ps_mm, ps_tr, ps_st) = pools
    (q_r3, c_r3, w_r3, cw_r3, qw_r3, out_r3) = aps

    # ---- loads (small tensors first so the PE can start early) ----
    w_r = rot8.tile([P, DT, D], f32r, name="w_f", tag="r8")
    nc.sync.dma_start(w_r, w_r3)

    q_r = qfam.tile([P, QT, D], f32r, name="q_f", tag="q_f")
    nc.sync.dma_start(q_r, q_r3)

    cwqw_r = statp.tile([P, DT, 2], f32r, name="cwqw_f", tag="cwqw_f")
    nc.gpsimd.dma_start(cwqw_r[:, :, 0:1], cw_r3)
    nc.gpsimd.dma_start(cwqw_r[:, :, 1:2], qw_r3)
    cwqw_f = cwqw_r.bitcast(f32)

    ctx_r = ctxp.tile([P, CT, D], f32r, name="ctx_f", tag="ctx_f")
    for g in range(4):
        nc.sync.dma_start(ctx_r[:, ds(g * 4, 4), :], c_r3[:, ds(g * 4, 4), :])
    ctx_f = ctx_r.bitcast(f32)

    # ---- constants ----
    id_f = constp.tile([P, P], f32, name="id_f", tag="id_f")
    from concourse.masks import make_identity
    make_identity(nc, id_f)
    id_r = constp.tile([P, P], f32r, name="id_r", tag="id_r")
    nc.vector.tensor_copy(id_r, id_f)
    ones2_f = constp.tile([1, 2], f32, name="ones2_f", tag="ones2_f")
    nc.vector.memset(ones2_f, 1.0)
    ones_row_f = constp.tile([1, 512], f32, name="ones_row_f", tag="ones_row_f")
    nc.vector.memset(ones_row_f, 1.0)
    ones_row_r = constp.tile([1, 512], f32r, name="ones_row", tag="ones_row")
    nc.vector.tensor_copy(ones_row_r, ones_row_f)

    def packed_transpose(dst_ap, srcs, evict_engine, accum_out=None):
        """len(srcs) transposes into one PSUM bank, one wide eviction."""
        n = len(srcs)
        ps = ps_tr.tile([P, 512], f32r, name="ptr", tag="tr")
        for j, src in enumerate(srcs):
            nc.tensor.matmul(ps[:, ds(j * P, P)], src, id_r,
                             is_transpose=True, start=(j == 0),
                             stop=(j == n - 1))
        if accum_out is not None:
            nc.scalar.activation(dst_ap, ps[:, 0: n * P], Copy,
                                 accum_out=accum_out)
        elif evict_engine == "act":
            nc.scalar.copy(dst_ap, ps[:, 0: n * P])
        else:
            nc.vector.tensor_copy(dst_ap, ps[:, 0: n * P])

    # ---- wT [e,d] and qT [e,q] (w arrives first) ----
    wT = rot8.tile([P, DT, D], f32r, name="wT", tag="r8")
    for eb in range(DT):
        packed_transpose(
            wT[:, eb, :],
            [w_r[:, dt, ds(eb * P, P)] for dt in range(DT)], "dve")
    qT = rot8.tile([P, DT, Q], f32r, name="qT", tag="r8")
    for eb in range(DT):
        packed_transpose(
            qT[:, eb, :],
            [q_r[:, qt, ds(eb * P, P)] for qt in range(QT)], "dve")

    # ---- H[d,q] = sum_e wT[e,d] qT[e,q]  (the small D x Q product) ----
    H = rot8.tile([P, DT, Q], f32r, name="H", tag="r8")
    for dtile in range(DT):
        pm = ps_mm.tile([P, 512], f32, name="pm", tag="mm")
        for eb in range(DT):
            nc.tensor.matmul(pm, wT[:, eb, ds(dtile * P, P)], qT[:, eb, :],
                             start=(eb == 0), stop=(eb == DT - 1))
        # H' = H + cw_w[d]: folds the cw[c] similarity term into the
        # contraction (sum_d ctx[c,d] cw_w[d] = cw[c]) at zero extra cost
        nc.scalar.activation(H[:, dtile, :], pm,
                             mybir.ActivationFunctionType.Identity,
                             bias=cwqw_f[:, dtile, 0:1], scale=1.0)

    # ---- qw_row [1, Q] ----
    qw_row = statp.tile([1, Q], f32r, name="qw_row", tag="qw_row")
    pqw = ps_st.tile([1, Q], f32, name="pst", tag="st")
    for dt in range(DT):
        nc.tensor.matmul(pqw, cwqw_r[:, dt, 1:2], qT[:, dt, :],
                         start=(dt == 0), stop=(dt == DT - 1))
    nc.vector.tensor_copy(qw_row, pqw)

    # ---- per ctx chunk: cT transposes, then raw sim for its cts ----
    cT = bigp.tile([P, DT, C], f32r, name="cT", tag="big")
    sim_r = simp.tile([P, CT, Q], f32, name="sim_t", tag="sim_t")
    F_r = fpool.tile([P, CT, Q], f32r, name="F_t", tag="F_t")
    F_t = F_r
    negm = statp.tile([P, CT], f32, name="negm", tag="negm")
    s_col = statp.tile([P, CT], f32, name="s_col", tag="s_col")

    def emit_cT_chunk(g):
        for dt in range(DT):
            packed_transpose(
                cT[:, dt, ds(g * 512, 512)],
                [ctx_r[:, 4 * g + j, ds(dt * P, P)] for j in range(4)], "dve")

    def emit_sim_chunk(g):
        for ct in range(4 * g, 4 * g + 4):
            # the exact-context output block goes straight from SBUF, on
            # the gpsimd queue so it never delays the sync input queue
            nc.gpsimd.dma_start(out_r3[:, ct, 0:D], ctx_f[:, ct, :])
            pm = ps_mm.tile([P, 512], f32, name="pm", tag="mm")
            for dt in range(DT):
                nc.tensor.matmul(pm, cT[:, dt, ds(ct * P, P)], H[:, dt, :],
                                 start=(dt == 0), stop=False)
            nc.tensor.matmul(pm, ones_row_r[0:1, 0:P], qw_row[0:1, :],
                             start=False, stop=True)
            nc.vector.tensor_reduce(negm[:, ds(ct, 1)], pm, axis=AxX,
                                    op=Max, negate=True)
            nc.scalar.copy(sim_r[:, ct, :], pm)

    # transposes of chunk g+1 are emitted before sim of chunk g so the PE
    # never catches up with the ctx input DMA
    emit_cT_chunk(0)
    emit_cT_chunk(1)
    emit_sim_chunk(0)
    emit_cT_chunk(2)
    emit_sim_chunk(1)
    emit_cT_chunk(3)
    emit_sim_chunk(2)
    emit_sim_chunk(3)

    # ---- global max K; negK_col = -K broadcast down the partitions ----
    nkp = statp.tile([P, 1], f32, name="nkp", tag="nkp")
    nc.vector.tensor_reduce(nkp, negm, axis=AxX, op=Min)  # -max_q per row
    pkt = ps_st.tile([1, P], f32, name="pst", tag="st")
    nc.tensor.transpose(pkt, nkp, id_f)
    negK11 = statp.tile([1, 1], f32, name="negK11", tag="negK11")
    nc.vector.tensor_reduce(negK11, pkt, axis=AxX, op=Min)  # -K
    # shift by +60: F = exp(sim - K + 60). The shift cancels in both
    # softmax normalizations but keeps the worst row sum (~e^{-90+60})
    # far from the f32 range floor, so 1/s_col cannot overflow to inf.
    krow = statp.tile([1, P], f32, name="krow", tag="krow")
    nc.vector.tensor_scalar_add(krow, negK11.to_broadcast([1, P]), 60.0)
    pkb = ps_st.tile([P, 2], f32, name="pst", tag="st")
    nc.tensor.matmul(pkb, krow, ones2_f, start=True, stop=True)
    negK_col = statp.tile([P, 1], f32, name="negK_col", tag="negK_col")
    nc.vector.tensor_copy(negK_col, pkb[:, 0:1])

    # ---- F = exp(sim - K); row sums via the act accumulator ----
    # scale=1.0 only: negative activation scale miscomputes on HW
    for ct in range(CT):
        nc.scalar.activation(F_t[:, ct, :], sim_r[:, ct, :], Exp,
                             bias=negK_col, scale=1.0,
                             accum_out=s_col[:, ds(ct, 1)])
    r_col = statp.tile([P, CT], f32, name="r_col", tag="r_col")
    nc.vector.reciprocal(r_col, s_col)

    # ---- ET [q,c] = F^T; evictions accumulate S_q[q] = sum_c F[c,q] ----
    ET = bigp.tile([P, QT, C], f32r, name="ET", tag="big")
    sqp = statp.tile([P, QT, 4], f32, name="sqp", tag="sqp")
    for g in range(4):
        for qt in range(QT):
            packed_transpose(
                ET[:, qt, ds(g * 512, 512)],
                [F_r[:, 4 * g + j, ds(qt * P, P)] for j in range(QT)],
                "act", accum_out=sqp[:, qt, ds(g, 1)])
    sq = statp.tile([P, QT], f32, name="sq", tag="sq")
    for qt in range(QT):
        nc.vector.tensor_reduce(sq[:, ds(qt, 1)], sqp[:, qt, :], axis=AxX,
                                op=mybir.AluOpType.add)
    rq = statp.tile([P, QT], f32, name="rq", tag="rq")
    nc.vector.reciprocal(rq, sq)

    # ---- G[q,d] = (1/S_q) sum_c F[c,q] ctx[c,d] ----
    G_r = rot8.tile([P, QT, D], f32r, name="G_f", tag="r8")
    G_f = G_r
    for qt in range(QT):
        pm = ps_mm.tile([P, 512], f32, name="pm", tag="mm")
        for ct in range(CT):
            nc.tensor.matmul(pm, F_r[:, ct, ds(qt * P, P)], ctx_r[:, ct, :],
                             start=(ct == 0), stop=(ct == CT - 1))
        nc.scalar.mul(G_f[:, qt, :], pm, rq[:, ds(qt, 1)])

    # ---- outputs per ct ----
    for ct in range(CT):
        pc = ps_mm.tile([P, 512], f32, name="pm", tag="mm")
        for qt in range(QT):
            nc.tensor.matmul(pc, ET[:, qt, ds(ct * P, P)], q_r[:, qt, :],
                             start=(qt == 0), stop=(qt == QT - 1))
        pq = ps_mm.tile([P, 512], f32, name="pm", tag="mm")
        for qt in range(QT):
            nc.tensor.matmul(pq, ET[:, qt, ds(ct * P, P)], G_r[:, qt, :],
                             start=(qt == 0), stop=(qt == QT - 1))
        st = stagep.tile([P, 3 * D], f32, name="st", tag="stage")
        # three writers on independent engines, no intra-tile chains,
        # everything built straight from PSUM
        nc.scalar.mul(st[:, 0:D], pc, r_col[:, ds(ct, 1)])
        nc.vector.scalar_tensor_tensor(st[:, ds(D, D)], pc,
                                       r_col[:, ds(ct, 1)], ctx_f[:, ct, :],
                                       op0=Mult, op1=Mult)
        nc.vector.scalar_tensor_tensor(st[:, ds(2 * D, D)], pq,
                                       r_col[:, ds(ct, 1)], ctx_f[:, ct, :],
                                       op0=Mult, op1=Mult)
        dma_eng = nc.sync if ct % 2 == 0 else nc.gpsimd
        dma_eng.dma_start(out_r3[:, ct, ds(D, 3 * D)], st)


def _build_bass(loop_n=1):
    import concourse.bass as bass  # noqa: F401
    import concourse.mybir as mybir
    import concourse.tile as tile
    from concourse import bacc

    f32 = mybir.dt.float32

    f32r = mybir.dt.float32r
    nc = bacc.Bacc("TRN2", debug=False, num_devices=N_CORES)
    q_d = nc.dram_tensor("query", [Q, D], f32r, kind="ExternalInput")
    c_d = nc.dram_tensor("context", [C, D], f32r, kind="ExternalInput")
    qw_d = nc.dram_tensor("query_weights", [D, 1], f32r, kind="ExternalInput")
    cw_d = nc.dram_tensor("context_weights", [D, 1], f32r, kind="ExternalInput")
    w_d = nc.dram_tensor("dot_weights", [D, D], f32r, kind="ExternalInput")
    out_d = nc.dram_tensor("out", [C, 4 * D], f32, kind="ExternalOutput")

    aps = (
        q_d.ap().rearrange("(t p) d -> p t d", p=P),
        c_d.ap().rearrange("(t p) d -> p t d", p=P),
        w_d.ap().rearrange("(t p) e -> p t e", p=P),
        cw_d.ap().rearrange("(t p) o -> p t o", p=P),
        qw_d.ap().rearrange("(t p) o -> p t o", p=P),
        out_d.ap().rearrange("(t p) f -> p t f", p=P),
    )

    with tile.TileContext(nc) as tc:
        with (
            tc.tile_pool(name="const", bufs=1) as constp,
            tc.tile_pool(name="stats", bufs=1) as statp,
            tc.tile_pool(name="ctxp", bufs=1) as ctxp,
            tc.tile_pool(name="qfam", bufs=1) as qfam,
            tc.tile_pool(name="rot8", bufs=2) as rot8,
            tc.tile_pool(name="fpool", bufs=1) as fpool,
            tc.tile_pool(name="simp", bufs=1) as simp,
            tc.tile_pool(name="big", bufs=2) as bigp,
            tc.tile_pool(name="stage", bufs=2) as stagep,
            tc.tile_pool(name="ps_mm", bufs=4, space="PSUM") as ps_mm,
            tc.tile_pool(name="ps_tr", bufs=3, space="PSUM") as ps_tr,
            tc.tile_pool(name="ps_st", bufs=1, space="PSUM") as ps_st,
        ):
            pools = (constp, statp, ctxp, qfam, rot8, fpool, simp, bigp,
                     stagep, ps_mm, ps_tr, ps_st)
            if loop_n > 1:
                # unroll several bodies per loop iteration: the For_i
                # all-engine barrier is expensive on this runtime, so
                # amortize it while keeping exactly loop_n body runs
                k = 8 if loop_n % 8 == 0 else (4 if loop_n % 4 == 0 else 1)
                with tc.For_i(0, loop_n // k, 1):
                    for _ in range(k):
                        _emit_body(nc, tc, pools, aps)
            else:
                _emit_body(nc, tc, pools, aps)
    nc.compile()
    return nc


def get_nc(loop_n=1):
    if loop_n not in _NC_CACHE:
        _NC_CACHE[loop_n] = _build_bass(loop_n)
    return _NC_CACHE[loop_n]


def kernel(query, context, query_weights, context_weights, dot_weights,
           mask=None):
    from concourse.bass_utils import run_bass_kernel_spmd

    query = np.ascontiguousarray(np.asarray(query, dtype=np.float32))
    context = np.ascontiguousarray(np.asarray(context, dtype=np.float32))
    query_weights = np.ascontiguousarray(np.asarray(query_weights, dtype=np.float32))
    context_weights = np.ascontiguousarray(np.asarray(context_weights, dtype=np.float32))
    dot_weights = np.ascontiguousarray(np.asarray(dot_weights, dtype=np.float32))
    # mask is all-True per the problem spec; NEG_INF * (~mask) == 0, so it
    # drops out of the computation entirely.

    nc = get_nc()
    in_maps = [
        {
            "query": query[b],
            "context": context[b],
            "query_weights": query_weights,
            "context_weights": context_weights,
            "dot_weights": dot_weights,
        }
        for b in range(B)
    ]
    res = run_bass_kernel_spmd(nc, in_maps, core_ids=list(range(N_CORES)))
    out = np.stack([res.results[b]["out"] for b in range(B)], axis=0)
    return np.ascontiguousarray(out.astype(np.float32))


if __name__ == "__main__":
    rng = np.random.default_rng(0)
    inputs = {
        "query": rng.standard_normal((B, Q, D), dtype=np.float32),
        "context": rng.standard_normal((B, C, D), dtype=np.float32),
        "query_weights": rng.standard_normal((D, 1), dtype=np.float32) * 0.05,
        "context_weights": rng.standard_normal((D, 1), dtype=np.float32) * 0.05,
        "dot_weights": rng.standard_normal((D, D), dtype=np.float32) * 0.05,
        "mask": np.ones((B, C, Q), dtype=bool),
    }
    out = kernel(**inputs)
    print("out", out.shape, out.dtype)


# revision 6
# speedup vs baseline: 14.0804x; 1.1916x over previous
"""ContextQueryAttention (BiDAF-style) Trainium2 kernel, v5.

Problem: nn_ContextQueryAttention_44066364457466
  query [B=8, Q=512, D=512], context [B=8, C=2048, D=512],
  query_weights/context_weights [D,1], dot_weights [D,D], mask all-True.
  out [B, C, 4D]: concat(context, c2q@query, context*that, context*qtc)

Sharding: data-parallel over batch. B == 8 == n_cores, one batch element
per NeuronCore, no collectives.

Math (per batch element; mask all-ones so it drops out):
  H[d,q]  = sum_e W[d,e] qT[e,q]       (contract the small side first:
  sim[c,q]= sum_d ctx[c,d] H[d,q] + cw[c] + qw[q]   saves 400M MACs vs
                                                    the (ctx@W)@qT order)
  Two-pass softmax around one global shift K = max sim:
    pass 1: evict raw sim to SBUF (f32), row maxes m_c on the fly
    K = max_c m_c  (tiny)
    pass 2: F = exp(sim - K + 60), row sums s_col via the accumulator.
    The +60 keeps the worst row sum (~e^{-90+60}) far enough above the
    f32 floor that 1/s_col cannot overflow; the shift cancels in both
    softmax normalizations.
  Both softmaxes come from this single F:
    c2q = F / rowsum(F)      q2c = F / colsum(F)
  ctq = c2q @ query;  G[q,d] = q2c^T @ ctx;  qtc = c2q @ G

Precision split: the sim chain (transposes, H, sim) runs f32r - exp is
exponentially sensitive to absolute logit error. Everything downstream
of exp works on attention weights in [0, e^60] where bf16's 0.4%
relative error washes out across 512-2048-term reductions: F, ET, G,
and the ctx/query copies feeding those matmuls are bf16 (PE does not
support mixed-dtype operands, so rhs-side bf16 casts of ctx/query are
made once per body).

Implementation notes:
- f32r via bitcast for the sim-side matmuls (PE rounds fp32 on ingest).
- Four 128x128 transposes are packed per PSUM bank, drained by one wide
  eviction; the bf16 ET evictions also produce the q2c column sums S_q
  via the activation accumulator, directly in the [q-part, QT] layout
  that G's eviction scale needs.
- cw enters sim through the H eviction bias; qw through one K=1 rank-1
  matmul per c-tile.
- DMA queues: inputs + half the ctx output copies on sync, other half
  on gpsimd; staged output blocks are written by their producer engines
  and DMA'd from those same queues (scalar: ctq block, vector: the two
  ctx* blocks) so every output DMA is self-ordered behind its producer
  and never cross-blocks another queue.
- G gets its own pool so next-body wT transposes reuse H's slot (freed
  mid-body) instead of G's (freed only at body end) - removes the
  body-boundary PE stall.
- Constants are emitted once, outside the repeat loop.
"""

import numpy as np

B, Q, C, D = 8, 512, 2048, 512
P = 128
QT, CT, DT = Q // P, C // P, D // P  # 4, 16, 4
N_CORES = 8

_NC_CACHE = {}


def ds(start, size):
    return slice(start, start + size)


def _emit_consts(nc, constp):
    import concourse.mybir as mybir

    f32 = mybir.dt.float32
    f32r = mybir.dt.float32r
    bf16 = mybir.dt.bfloat16

    id_f = constp.tile([P, P], f32, name="id_f", tag="id_f")
    from concourse.masks import make_identity
    make_identity(nc, id_f)
    id_r = constp.tile([P, P], f32r, name="id_r", tag="id_r")
    nc.vector.tensor_copy(id_r, id_f)
    id_b = constp.tile([P, P], bf16, name="id_b", tag="id_b")
    nc.vector.tensor_copy(id_b, id_f)
    ones2_f = constp.tile([1, 2], f32, name="ones2_f", tag="ones2_f")
    nc.vector.memset(ones2_f, 1.0)
    ones_row_f = constp.tile([1, 512], f32, name="ones_row_f", tag="ones_row_f")
    nc.vector.memset(ones_row_f, 1.0)
    ones_row_r = constp.tile([1, 512], f32r, name="ones_row", tag="ones_row")
    nc.vector.tensor_copy(ones_row_r, ones_row_f)
    return (id_f, id_r, id_b, ones2_f, ones_row_r)


def _emit_body(nc, tc, pools, aps, consts):
    import concourse.mybir as mybir

    f32 = mybir.dt.float32
    f32r = mybir.dt.float32r
    bf16 = mybir.dt.bfloat16
    Exp = mybir.ActivationFunctionType.Exp
    Copy = mybir.ActivationFunctionType.Copy
    Mult = mybir.AluOpType.mult
    Max = mybir.AluOpType.max
    Min = mybir.AluOpType.min
    AxX = mybir.AxisListType.X

    (statp, ctxp, cbfp, qfam, qbfp, rot8, gpool, fpool, simp, cTp, ETp,
     stagep, ps_mm, ps_tr, ps_st) = pools
    (q_r3, c_r3, w_r3, cw_r3, qw_r3, out_r3) = aps
    (id_f, id_r, id_b, ones2_f, ones_row_r) = consts

    # ---- loads (small tensors first so the PE can start early) ----
    w_r = rot8.tile([P, DT, D], f32r, name="w_f", tag="r8")
    nc.sync.dma_start(w_r, w_r3)

    q_r = qfam.tile([P, QT, D], f32r, name="q_f", tag="q_f")
    nc.sync.dma_start(q_r, q_r3)

    cwqw_r = statp.tile([P, DT, 2], f32r, name="cwqw_f", tag="cwqw_f")
    nc.gpsimd.dma_start(cwqw_r[:, :, 0:1], cw_r3)
    nc.gpsimd.dma_start(cwqw_r[:, :, 1:2], qw_r3)
    cwqw_f = cwqw_r.bitcast(f32)

    ctx_r = ctxp.tile([P, CT, D], f32r, name="ctx_f", tag="ctx_f")
    for g in range(4):
        nc.sync.dma_start(ctx_r[:, ds(g * 4, 4), :], c_r3[:, ds(g * 4, 4), :])
    ctx_f = ctx_r.bitcast(f32)

    # bf16 copies feeding the post-softmax matmul rhs sides
    q_bf = qbfp.tile([P, QT, D], bf16, name="q_bf", tag="q_bf")
    nc.gpsimd.tensor_copy(q_bf, q_r.bitcast(f32))
    ctx_bf = cbfp.tile([P, CT, D], bf16, name="ctx_bf", tag="ctx_bf")
    for g in range(4):
        nc.gpsimd.tensor_copy(ctx_bf[:, ds(g * 4, 4), :],
                              ctx_f[:, ds(g * 4, 4), :])

    def packed_transpose(dst_ap, srcs, evict_engine, accum_out=None,
                         ident=id_r, psdt=f32r):
        """len(srcs) transposes into one PSUM bank, one wide eviction."""
        n = len(srcs)
        ps = ps_tr.tile([P, 512], psdt, name="ptr", tag="tr")
        for j, src in enumerate(srcs):
            nc.tensor.matmul(ps[:, ds(j * P, P)], src, ident,
                             is_transpose=True, start=(j == 0),
                             stop=(j == n - 1))
        if accum_out is not None:
            nc.scalar.activation(dst_ap, ps[:, 0: n * P], Copy,
                                 accum_out=accum_out)
        elif evict_engine == "act":
            nc.scalar.copy(dst_ap, ps[:, 0: n * P])
        else:
            nc.vector.tensor_copy(dst_ap, ps[:, 0: n * P])

    # ---- wT [e,d] and qT [e,q] (w arrives first) ----
    wT = rot8.tile([P, DT, D], f32r, name="wT", tag="r8")
    for eb in range(DT):
        packed_transpose(
            wT[:, eb, :],
            [w_r[:, dt, ds(eb * P, P)] for dt in range(DT)], "dve")
    qT = rot8.tile([P, DT, Q], f32r, name="qT", tag="r8")
    for eb in range(DT):
        packed_transpose(
            qT[:, eb, :],
            [q_r[:, qt, ds(eb * P, P)] for qt in range(QT)], "dve")

    # ---- H[d,q] = sum_e wT[e,d] qT[e,q]  (the small D x Q product) ----
    H = rot8.tile([P, DT, Q], f32r, name="H", tag="r8")
    for dtile in range(DT):
        pm = ps_mm.tile([P, 512], f32, name="pm", tag="mm")
        for eb in range(DT):
            nc.tensor.matmul(pm, wT[:, eb, ds(dtile * P, P)], qT[:, eb, :],
                             start=(eb == 0), stop=(eb == DT - 1))
        # H' = H + cw_w[d]: folds the cw[c] similarity term into the
        # contraction (sum_d ctx[c,d] cw_w[d] = cw[c]) at zero extra cost
        nc.scalar.activation(H[:, dtile, :], pm,
                             mybir.ActivationFunctionType.Identity,
                             bias=cwqw_f[:, dtile, 0:1], scale=1.0)

    # ---- qw_row [1, Q] ----
    qw_row = statp.tile([1, Q], f32r, name="qw_row", tag="qw_row")
    pqw = ps_st.tile([1, Q], f32, name="pst", tag="st")
    for dt in range(DT):
        nc.tensor.matmul(pqw, cwqw_r[:, dt, 1:2], qT[:, dt, :],
                         start=(dt == 0), stop=(dt == DT - 1))
    nc.vector.tensor_copy(qw_row, pqw)

    # ---- per ctx chunk: cT transposes, then raw sim for its cts ----
    cT = cTp.tile([P, DT, C], f32r, name="cT", tag="cT")
    sim_r = simp.tile([P, CT, Q], f32, name="sim_t", tag="sim_t")
    F_t = fpool.tile([P, CT, Q], bf16, name="F_t", tag="F_t")
    negm = statp.tile([P, CT], f32, name="negm", tag="negm")
    s_col = statp.tile([P, CT], f32, name="s_col", tag="s_col")

    def emit_cT_chunk(g):
        for dt in range(DT):
            packed_transpose(
                cT[:, dt, ds(g * 512, 512)],
                [ctx_r[:, 4 * g + j, ds(dt * P, P)] for j in range(4)], "dve")

    def emit_sim_chunk(g):
        for ct in range(4 * g, 4 * g + 4):
            # the exact-context output block goes straight from SBUF; no
            # compute dependencies, so these never block their queue
            dma_eng = nc.sync if ct % 2 == 0 else nc.gpsimd
            dma_eng.dma_start(out_r3[:, ct, 0:D], ctx_f[:, ct, :])
            pm = ps_mm.tile([P, 512], f32, name="pm", tag="mm")
            for dt in range(DT):
                nc.tensor.matmul(pm, cT[:, dt, ds(ct * P, P)], H[:, dt, :],
                                 start=(dt == 0), stop=False)
            nc.tensor.matmul(pm, ones_row_r[0:1, 0:P], qw_row[0:1, :],
                             start=False, stop=True)
            nc.vector.tensor_reduce(negm[:, ds(ct, 1)], pm, axis=AxX,
                                    op=Max, negate=True)
            nc.scalar.copy(sim_r[:, ct, :], pm)

    # transposes of chunk g+1 are emitted before sim of chunk g so the PE
    # never catches up with the ctx input DMA
    emit_cT_chunk(0)
    emit_cT_chunk(1)
    emit_sim_chunk(0)
    emit_cT_chunk(2)
    emit_sim_chunk(1)
    emit_cT_chunk(3)
    emit_sim_chunk(2)
    emit_sim_chunk(3)

    # ---- global max K; negK_col = (60 - K) broadcast down partitions ----
    nkp = statp.tile([P, 1], f32, name="nkp", tag="nkp")
    nc.vector.tensor_reduce(nkp, negm, axis=AxX, op=Min)  # -max_q per row
    pkt = ps_st.tile([1, P], f32, name="pst", tag="st")
    nc.tensor.transpose(pkt, nkp, id_f)
    negK11 = statp.tile([1, 1], f32, name="negK11", tag="negK11")
    nc.vector.tensor_reduce(negK11, pkt, axis=AxX, op=Min)  # -K
    # shift by +60: F = exp(sim - K + 60). The shift cancels in both
    # softmax normalizations but keeps the worst row sum (~e^{-90+60})
    # far from the f32 range floor, so 1/s_col cannot overflow to inf.
    krow = statp.tile([1, P], f32, name="krow", tag="krow")
    nc.vector.tensor_scalar_add(krow, negK11.to_broadcast([1, P]), 60.0)
    pkb = ps_st.tile([P, 2], f32, name="pst", tag="st")
    nc.tensor.matmul(pkb, krow, ones2_f, start=True, stop=True)
    negK_col = statp.tile([P, 1], f32, name="negK_col", tag="negK_col")
    nc.vector.tensor_copy(negK_col, pkb[:, 0:1])

    # ---- F = exp(sim - K + 60) (bf16); row sums via the accumulator;
    # ET [q,c] = F^T, its evictions accumulate S_q[q] = sum_c F[c,q].
    # exp of chunk g and ET transposes of chunk g are interleaved so the
    # PE restarts as soon as the first four F tiles exist. ----
    ET = ETp.tile([P, QT, C], bf16, name="ET", tag="ET")
    sqp = statp.tile([P, QT, 4], f32, name="sqp", tag="sqp")
    for g in range(4):
        for ct in range(4 * g, 4 * g + 4):
            # scale=1.0 only: negative activation scale miscomputes on HW
            nc.scalar.activation(F_t[:, ct, :], sim_r[:, ct, :], Exp,
                                 bias=negK_col, scale=1.0,
                                 accum_out=s_col[:, ds(ct, 1)])
        for qt in range(QT):
            packed_transpose(
                ET[:, qt, ds(g * 512, 512)],
                [F_t[:, 4 * g + j, ds(qt * P, P)] for j in range(4)],
                "act", accum_out=sqp[:, qt, ds(g, 1)],
                ident=id_b, psdt=bf16)
    r_col = statp.tile([P, CT], f32, name="r_col", tag="r_col")
    nc.vector.reciprocal(r_col, s_col)
    sq = statp.tile([P, QT], f32, name="sq", tag="sq")
    for qt in range(QT):
        nc.vector.tensor_reduce(sq[:, ds(qt, 1)], sqp[:, qt, :], axis=AxX,
                                op=mybir.AluOpType.add)
    rq = statp.tile([P, QT], f32, name="rq", tag="rq")
    nc.vector.reciprocal(rq, sq)

    # ---- G[q,d] = (1/S_q) sum_c F[c,q] ctx[c,d] ----
    G_b = gpool.tile([P, QT, D], bf16, name="G_b", tag="G_b")
    for qt in range(QT):
        pm = ps_mm.tile([P, 512], f32, name="pm", tag="mm")
        for ct in range(CT):
            nc.tensor.matmul(pm, F_t[:, ct, ds(qt * P, P)],
                             ctx_bf[:, ct, :],
                             start=(ct == 0), stop=(ct == CT - 1))
        nc.scalar.mul(G_b[:, qt, :], pm, rq[:, ds(qt, 1)])

    # ---- outputs per ct ----
    for ct in range(CT):
        pc = ps_mm.tile([P, 512], f32, name="pm", tag="mm")
        for qt in range(QT):
            nc.tensor.matmul(pc, ET[:, qt, ds(ct * P, P)], q_bf[:, qt, :],
                             start=(qt == 0), stop=(qt == QT - 1))
        pq = ps_mm.tile([P, 512], f32, name="pm", tag="mm")
        for qt in range(QT):
            nc.tensor.matmul(pq, ET[:, qt, ds(ct * P, P)], G_b[:, qt, :],
                             start=(qt == 0), stop=(qt == QT - 1))
        st = stagep.tile([P, 3 * D], f32, name="st", tag="stage")
        # three writers on independent engines, each queue DMAs the block
        # its own engine produced, so the DMAs are self-ordered
        nc.scalar.mul(st[:, 0:D], pc, r_col[:, ds(ct, 1)])
        nc.scalar.dma_start(out_r3[:, ct, ds(D, D)], st[:, 0:D])
        nc.vector.scalar_tensor_tensor(st[:, ds(D, D)], pc,
                                       r_col[:, ds(ct, 1)], ctx_f[:, ct, :],
                                       op0=Mult, op1=Mult)
        nc.vector.scalar_tensor_tensor(st[:, ds(2 * D, D)], pq,
                                       r_col[:, ds(ct, 1)], ctx_f[:, ct, :],
                                       op0=Mult, op1=Mult)
        dma_eng = nc.sync if ct % 2 == 0 else nc.gpsimd
        dma_eng.dma_start(out_r3[:, ct, ds(2 * D, 2 * D)], st[:, ds(D, 2 * D)])


def _build_bass(loop_n=1):
    import concourse.bass as bass  # noqa: F401
    import concourse.mybir as mybir
    import concourse.tile as tile
    from concourse import bacc

    f32 = mybir.dt.float32

    f32r = mybir.dt.float32r
    nc = bacc.Bacc("TRN2", debug=False, num_devices=N_CORES)
    q_d = nc.dram_tensor("query", [Q, D], f32r, kind="ExternalInput")
    c_d = nc.dram_tensor("context", [C, D], f32r, kind="ExternalInput")
    qw_d = nc.dram_tensor("query_weights", [D, 1], f32r, kind="ExternalInput")
    cw_d = nc.dram_tensor("context_weights", [D, 1], f32r, kind="ExternalInput")
    w_d = nc.dram_tensor("dot_weights", [D, D], f32r, kind="ExternalInput")
    out_d = nc.dram_tensor("out", [C, 4 * D], f32, kind="ExternalOutput")

    aps = (
        q_d.ap().rearrange("(t p) d -> p t d", p=P),
        c_d.ap().rearrange("(t p) d -> p t d", p=P),
        w_d.ap().rearrange("(t p) e -> p t e", p=P),
        cw_d.ap().rearrange("(t p) o -> p t o", p=P),
        qw_d.ap().rearrange("(t p) o -> p t o", p=P),
        out_d.ap().rearrange("(t p) f -> p t f", p=P),
    )

    with tile.TileContext(nc) as tc:
        with (
            tc.tile_pool(name="const", bufs=1) as constp,
            tc.tile_pool(name="stats", bufs=1) as statp,
            tc.tile_pool(name="ctxp", bufs=1) as ctxp,
            tc.tile_pool(name="cbf", bufs=1) as cbfp,
            tc.tile_pool(name="qfam", bufs=1) as qfam,
            tc.tile_pool(name="qbf", bufs=1) as qbfp,
            tc.tile_pool(name="rot8", bufs=2) as rot8,
            tc.tile_pool(name="gpool", bufs=2) as gpool,
            tc.tile_pool(name="fpool", bufs=1) as fpool,
            tc.tile_pool(name="simp", bufs=1) as simp,
            tc.tile_pool(name="cTp", bufs=1) as cTp,
            tc.tile_pool(name="ETp", bufs=1) as ETp,
            tc.tile_pool(name="stage", bufs=3) as stagep,
            tc.tile_pool(name="ps_mm", bufs=4, space="PSUM") as ps_mm,
            tc.tile_pool(name="ps_tr", bufs=3, space="PSUM") as ps_tr,
            tc.tile_pool(name="ps_st", bufs=1, space="PSUM") as ps_st,
        ):
            pools = (statp, ctxp, cbfp, qfam, qbfp, rot8, gpool, fpool,
                     simp, cTp, ETp, stagep, ps_mm, ps_tr, ps_st)
            consts = _emit_consts(nc, constp)
            if loop_n > 1:
                # unroll several bodies per loop iteration: the For_i
                # all-engine barrier is expensive on this runtime, so
                # amortize it while keeping exactly loop_n body runs
                k = 16 if loop_n % 16 == 0 else (
                    8 if loop_n % 8 == 0 else (4 if loop_n % 4 == 0 else 1))
                with tc.For_i(0, loop_n // k, 1):
                    for _ in range(k):
                        _emit_body(nc, tc, pools, aps, consts)
            else:
                _emit_body(nc, tc, pools, aps, consts)
    nc.compile()
    return nc


def get_nc(loop_n=1):
    if loop_n not in _NC_CACHE:
        _NC_CACHE[loop_n] = _build_bass(loop_n)
    return _NC_CACHE[loop_n]


def kernel(query, context, query_weights, context_weights, dot_weights,
           mask=None):
    from concourse.bass_utils import run_bass_kernel_spmd

    query = np.ascontiguousarray(np.asarray(query, dtype=np.float32))
    context = np.ascontiguousarray(np.asarray(context, dtype=np.float32))
    query_weights = np.ascontiguousarray(np.asarray(query_weights, dtype=np.float32))
    context_weights = np.ascontiguousarray(np.asarray(context_weights, dtype=np.float32))
    dot_weights = np.ascontiguousarray(np.asarray(dot_weights, dtype=np.float32))
    # mask is all-True per the problem spec; NEG_INF * (~mask) == 0, so it
    # drops out of the computation entirely.

    nc = get_nc()
    in_maps = [
        {
            "query": query[b],
            "context": context[b],
            "query_weights": query_weights,
            "context_weights": context_weights,
            "dot_weights": dot_weights,
        }
        for b in range(B)
    ]
    res = run_bass_kernel_spmd(nc, in_maps, core_ids=list(range(N_CORES)))
    out = np.stack([res.results[b]["out"] for b in range(B)], axis=0)
    return np.ascontiguousarray(out.astype(np.float32))


if __name__ == "__main__":
    rng = np.random.default_rng(0)
    inputs = {
        "query": rng.standard_normal((B, Q, D), dtype=np.float32),
        "context": rng.standard_normal((B, C, D), dtype=np.float32),
        "query_weights": rng.standard_normal((D, 1), dtype=np.float32) * 0.05,
        "context_weights": rng.standard_normal((D, 1), dtype=np.float32) * 0.05,
        "dot_weights": rng.standard_normal((D, D), dtype=np.float32) * 0.05,
        "mask": np.ones((B, C, Q), dtype=bool),
    }
    out = kernel(**inputs)
    print("out", out.shape, out.dtype)


# revision 10
# speedup vs baseline: 14.5660x; 1.0345x over previous
"""ContextQueryAttention (BiDAF-style) Trainium2 kernel, v5.

Problem: nn_ContextQueryAttention_44066364457466
  query [B=8, Q=512, D=512], context [B=8, C=2048, D=512],
  query_weights/context_weights [D,1], dot_weights [D,D], mask all-True.
  out [B, C, 4D]: concat(context, c2q@query, context*that, context*qtc)

Sharding: data-parallel over batch. B == 8 == n_cores, one batch element
per NeuronCore, no collectives.

Math (per batch element; mask all-ones so it drops out):
  H[d,q]  = sum_e W[d,e] qT[e,q]       (contract the small side first:
  sim[c,q]= sum_d ctx[c,d] H[d,q] + cw[c] + qw[q]   saves 400M MACs vs
                                                    the (ctx@W)@qT order)
  Two-pass softmax around one global shift K = max sim:
    pass 1: evict raw sim to SBUF (f32), row maxes m_c on the fly
    K = max_c m_c  (tiny)
    pass 2: F = exp(sim - K + 60), row sums s_col via the accumulator.
    The +60 keeps the worst row sum (~e^{-90+60}) far enough above the
    f32 floor that 1/s_col cannot overflow; the shift cancels in both
    softmax normalizations.
  Both softmaxes come from this single F:
    c2q = F / rowsum(F)      q2c = F / colsum(F)
  ctq = c2q @ query;  G[q,d] = q2c^T @ ctx;  qtc = c2q @ G

Precision split: the sim chain (transposes, H, sim) runs f32r - exp is
exponentially sensitive to absolute logit error. Everything downstream
of exp works on attention weights in [0, e^60] where bf16's 0.4%
relative error washes out across 512-2048-term reductions: F, ET, G,
and the ctx/query copies feeding those matmuls are bf16 (PE does not
support mixed-dtype operands, so rhs-side bf16 casts of ctx/query are
made once per body).

Implementation notes:
- f32r via bitcast for the sim-side matmuls (PE rounds fp32 on ingest).
- Four 128x128 transposes are packed per PSUM bank, drained by one wide
  eviction; the bf16 ET evictions also produce the q2c column sums S_q
  via the activation accumulator, directly in the [q-part, QT] layout
  that G's eviction scale needs.
- cw enters sim through the H eviction bias; qw through one K=1 rank-1
  matmul per c-tile.
- DMA queues: inputs + half the ctx output copies on sync, other half
  on gpsimd; staged output blocks are written by their producer engines
  and DMA'd from those same queues (scalar: ctq block, vector: the two
  ctx* blocks) so every output DMA is self-ordered behind its producer
  and never cross-blocks another queue.
- G gets its own pool so next-body wT transposes reuse H's slot (freed
  mid-body) instead of G's (freed only at body end) - removes the
  body-boundary PE stall.
- Constants are emitted once, outside the repeat loop.
"""

import numpy as np

B, Q, C, D = 8, 512, 2048, 512
P = 128
QT, CT, DT = Q // P, C // P, D // P  # 4, 16, 4
N_CORES = 8

_NC_CACHE = {}


def ds(start, size):
    return slice(start, start + size)


def _emit_consts(nc, constp):
    import concourse.mybir as mybir

    f32 = mybir.dt.float32
    f32r = mybir.dt.float32r
    bf16 = mybir.dt.bfloat16

    id_f = constp.tile([P, P], f32, name="id_f", tag="id_f")
    from concourse.masks import make_identity
    make_identity(nc, id_f)
    id_r = constp.tile([P, P], f32r, name="id_r", tag="id_r")
    nc.vector.tensor_copy(id_r, id_f)
    id_b = constp.tile([P, P], bf16, name="id_b", tag="id_b")
    nc.vector.tensor_copy(id_b, id_f)
    ones2_f = constp.tile([1, 2], f32, name="ones2_f", tag="ones2_f")
    nc.vector.memset(ones2_f, 1.0)
    ones_row_f = constp.tile([1, 512], f32, name="ones_row_f", tag="ones_row_f")
    nc.vector.memset(ones_row_f, 1.0)
    ones_row_r = constp.tile([1, 512], f32r, name="ones_row", tag="ones_row")
    nc.vector.tensor_copy(ones_row_r, ones_row_f)
    return (id_f, id_r, id_b, ones2_f, ones_row_r)


def _emit_body(nc, tc, pools, aps, consts):
    import concourse.mybir as mybir

    f32 = mybir.dt.float32
    f32r = mybir.dt.float32r
    bf16 = mybir.dt.bfloat16
    Exp = mybir.ActivationFunctionType.Exp
    Copy = mybir.ActivationFunctionType.Copy
    Mult = mybir.AluOpType.mult
    Max = mybir.AluOpType.max
    Min = mybir.AluOpType.min
    AxX = mybir.AxisListType.X

    (statp, ctxp, cbfp, qfam, qbfp, rot8, gpool, fpool, simp, cTp, ETp,
     stagep, ps_mm, ps_tr, ps_st) = pools
    (q_r3, c_r3, w_r3, cw_r3, qw_r3, out_r3) = aps
    (id_f, id_r, id_b, ones2_f, ones_row_r) = consts

    # ---- loads (small tensors first so the PE can start early) ----
    w_r = rot8.tile([P, DT, D], f32r, name="w_f", tag="r8")
    nc.sync.dma_start(w_r, w_r3)

    q_r = qfam.tile([P, QT, D], f32r, name="q_f", tag="q_f")
    nc.sync.dma_start(q_r, q_r3)

    cwqw_r = statp.tile([P, DT, 2], f32r, name="cwqw_f", tag="cwqw_f")
    nc.sync.dma_start(cwqw_r[:, :, 0:1], cw_r3)
    nc.sync.dma_start(cwqw_r[:, :, 1:2], qw_r3)
    cwqw_f = cwqw_r.bitcast(f32)

    ctx_r = ctxp.tile([P, CT, D], f32r, name="ctx_f", tag="ctx_f")
    for g in range(4):
        nc.sync.dma_start(ctx_r[:, ds(g * 4, 4), :], c_r3[:, ds(g * 4, 4), :])
    ctx_f = ctx_r.bitcast(f32)

    # bf16 copies feeding the post-softmax matmul rhs sides (DVE casts:
    # ~0.7us per [P,512] vs ~7us per chunk on gpsimd)
    q_bf = qbfp.tile([P, QT, D], bf16, name="q_bf", tag="q_bf")
    nc.vector.tensor_copy(q_bf, q_r.bitcast(f32))
    ctx_bf = cbfp.tile([P, CT, D], bf16, name="ctx_bf", tag="ctx_bf")
    for g in range(4):
        nc.vector.tensor_copy(ctx_bf[:, ds(g * 4, 4), :],
                              ctx_f[:, ds(g * 4, 4), :])

    def packed_transpose(dst_ap, srcs, evict_engine, accum_out=None,
                         ident=id_r, psdt=f32r):
        """len(srcs) transposes into one PSUM bank, one wide eviction."""
        n = len(srcs)
        ps = ps_tr.tile([P, 512], psdt, name="ptr", tag="tr")
        for j, src in enumerate(srcs):
            nc.tensor.matmul(ps[:, ds(j * P, P)], src, ident,
                             is_transpose=True, start=(j == 0),
                             stop=(j == n - 1))
        if accum_out is not None:
            nc.scalar.activation(dst_ap, ps[:, 0: n * P], Copy,
                                 accum_out=accum_out)
        elif evict_engine == "act":
            nc.scalar.copy(dst_ap, ps[:, 0: n * P])
        else:
            nc.vector.tensor_copy(dst_ap, ps[:, 0: n * P])

    # ---- wT [e,d] and qT [e,q] (w arrives first) ----
    wT = rot8.tile([P, DT, D], f32r, name="wT", tag="r8")
    for eb in range(DT):
        packed_transpose(
            wT[:, eb, :],
            [w_r[:, dt, ds(eb * P, P)] for dt in range(DT)], "act")
    qT = rot8.tile([P, DT, Q], f32r, name="qT", tag="r8")
    for eb in range(DT):
        packed_transpose(
            qT[:, eb, :],
            [q_r[:, qt, ds(eb * P, P)] for qt in range(QT)], "act")

    # ---- H[d,q] = sum_e wT[e,d] qT[e,q]  (the small D x Q product) ----
    H = rot8.tile([P, DT, Q], f32r, name="H", tag="r8")
    for dtile in range(DT):
        pm = ps_mm.tile([P, 512], f32, name="pm", tag="mm")
        for eb in range(DT):
            nc.tensor.matmul(pm, wT[:, eb, ds(dtile * P, P)], qT[:, eb, :],
                             start=(eb == 0), stop=(eb == DT - 1))
        # H' = H + cw_w[d]: folds the cw[c] similarity term into the
        # contraction (sum_d ctx[c,d] cw_w[d] = cw[c]) at zero extra cost
        nc.scalar.activation(H[:, dtile, :], pm,
                             mybir.ActivationFunctionType.Identity,
                             bias=cwqw_f[:, dtile, 0:1], scale=1.0)

    # ---- qw_row [1, Q] ----
    qw_row = statp.tile([1, Q], f32r, name="qw_row", tag="qw_row")
    pqw = ps_st.tile([1, Q], f32, name="pst", tag="st")
    for dt in range(DT):
        nc.tensor.matmul(pqw, cwqw_r[:, dt, 1:2], qT[:, dt, :],
                         start=(dt == 0), stop=(dt == DT - 1))
    nc.vector.tensor_copy(qw_row, pqw)

    # ---- per ctx chunk: cT transposes, then raw sim for its cts ----
    cT = cTp.tile([P, DT, C], f32r, name="cT", tag="cT")
    sim_r = simp.tile([P, CT, Q], f32, name="sim_t", tag="sim_t")
    F_t = fpool.tile([P, CT, Q], bf16, name="F_t", tag="F_t")
    negm = statp.tile([P, CT], f32, name="negm", tag="negm")
    s_col = statp.tile([P, CT], f32, name="s_col", tag="s_col")

    def emit_cT_chunk(g):
        for dt in range(DT):
            packed_transpose(
                cT[:, dt, ds(g * 512, 512)],
                [ctx_r[:, 4 * g + j, ds(dt * P, P)] for j in range(4)], "dve")

    def emit_sim_chunk(g):
        for ct in range(4 * g, 4 * g + 4):
            # the exact-context output block goes straight from SBUF; no
            # compute dependencies, so these drain at dispatch rate and
            # never block the input loads queued behind them
            nc.sync.dma_start(out_r3[:, ct, 0:D], ctx_f[:, ct, :])
            pm = ps_mm.tile([P, 512], f32, name="pm", tag="mm")
            for dt in range(DT):
                nc.tensor.matmul(pm, cT[:, dt, ds(ct * P, P)], H[:, dt, :],
                                 start=(dt == 0), stop=False)
            nc.tensor.matmul(pm, ones_row_r[0:1, 0:P], qw_row[0:1, :],
                             start=False, stop=True)
            nc.vector.tensor_reduce(negm[:, ds(ct, 1)], pm, axis=AxX,
                                    op=Max, negate=True)
            nc.scalar.copy(sim_r[:, ct, :], pm)

    # transposes of chunk g+1 are emitted before sim of chunk g so the PE
    # never catches up with the ctx input DMA
    emit_cT_chunk(0)
    emit_cT_chunk(1)
    emit_sim_chunk(0)
    emit_cT_chunk(2)
    emit_sim_chunk(1)
    emit_cT_chunk(3)
    emit_sim_chunk(2)
    emit_sim_chunk(3)

    # ---- global max K; negK_col = (60 - K) broadcast down partitions ----
    nkp = statp.tile([P, 1], f32, name="nkp", tag="nkp")
    nc.vector.tensor_reduce(nkp, negm, axis=AxX, op=Min)  # -max_q per row
    pkt = ps_st.tile([1, P], f32, name="pst", tag="st")
    nc.tensor.transpose(pkt, nkp, id_f)
    negK11 = statp.tile([1, 1], f32, name="negK11", tag="negK11")
    nc.vector.tensor_reduce(negK11, pkt, axis=AxX, op=Min)  # -K
    # shift by +60: F = exp(sim - K + 60). The shift cancels in both
    # softmax normalizations but keeps the worst row sum (~e^{-90+60})
    # far from the f32 range floor, so 1/s_col cannot overflow to inf.
    krow = statp.tile([1, P], f32, name="krow", tag="krow")
    nc.vector.tensor_scalar_add(krow, negK11.to_broadcast([1, P]), 60.0)
    pkb = ps_st.tile([P, 2], f32, name="pst", tag="st")
    nc.tensor.matmul(pkb, krow, ones2_f, start=True, stop=True)
    negK_col = statp.tile([P, 1], f32, name="negK_col", tag="negK_col")
    nc.vector.tensor_copy(negK_col, pkb[:, 0:1])

    # ---- F = exp(sim - K + 60) (bf16); row sums via the accumulator;
    # ET [q,c] = F^T, its evictions accumulate S_q[q] = sum_c F[c,q].
    # exp of chunk g and ET transposes of chunk g are interleaved so the
    # PE restarts as soon as the first four F tiles exist. ----
    ET = ETp.tile([P, QT, C], bf16, name="ET", tag="ET")
    sqp = statp.tile([P, QT, 4], f32, name="sqp", tag="sqp")
    for g in range(4):
        for ct in range(4 * g, 4 * g + 4):
            # scale=1.0 only: negative activation scale miscomputes on HW
            nc.scalar.activation(F_t[:, ct, :], sim_r[:, ct, :], Exp,
                                 bias=negK_col, scale=1.0,
                                 accum_out=s_col[:, ds(ct, 1)])
        for qt in range(QT):
            packed_transpose(
                ET[:, qt, ds(g * 512, 512)],
                [F_t[:, 4 * g + j, ds(qt * P, P)] for j in range(4)],
                "dve", ident=id_b, psdt=bf16)
            nc.vector.tensor_reduce(sqp[:, qt, ds(g, 1)],
                                    ET[:, qt, ds(g * 512, 512)], axis=AxX,
                                    op=mybir.AluOpType.add)
    r_col = statp.tile([P, CT], f32, name="r_col", tag="r_col")
    nc.vector.reciprocal(r_col, s_col)
    sq = statp.tile([P, QT], f32, name="sq", tag="sq")
    for qt in range(QT):
        nc.vector.tensor_reduce(sq[:, ds(qt, 1)], sqp[:, qt, :], axis=AxX,
                                op=mybir.AluOpType.add)
    rq = statp.tile([P, QT], f32, name="rq", tag="rq")
    nc.vector.reciprocal(rq, sq)

    # ---- G[q,d] = (1/S_q) sum_c F[c,q] ctx[c,d] ----
    G_b = gpool.tile([P, QT, D], bf16, name="G_b", tag="G_b")
    for qt in range(QT):
        pm = ps_mm.tile([P, 512], f32, name="pm", tag="mm")
        for ct in range(CT):
            nc.tensor.matmul(pm, F_t[:, ct, ds(qt * P, P)],
                             ctx_bf[:, ct, :],
                             start=(ct == 0), stop=(ct == CT - 1))
        nc.scalar.mul(G_b[:, qt, :], pm, rq[:, ds(qt, 1)])

    # ---- outputs per ct ----
    for ct in range(CT):
        pc = ps_mm.tile([P, 512], f32, name="pm", tag="mm")
        for qt in range(QT):
            nc.tensor.matmul(pc, ET[:, qt, ds(ct * P, P)], q_bf[:, qt, :],
                             start=(qt == 0), stop=(qt == QT - 1))
        pq = ps_mm.tile([P, 512], f32, name="pm", tag="mm")
        for qt in range(QT):
            nc.tensor.matmul(pq, ET[:, qt, ds(ct * P, P)], G_b[:, qt, :],
                             start=(qt == 0), stop=(qt == QT - 1))
        st = stagep.tile([P, 3 * D], f32, name="st", tag="stage")
        # three writers on independent engines, each queue DMAs the block
        # its own engine produced, so the DMAs are self-ordered
        nc.scalar.mul(st[:, 0:D], pc, r_col[:, ds(ct, 1)])
        nc.scalar.dma_start(out_r3[:, ct, ds(D, D)], st[:, 0:D])
        nc.vector.scalar_tensor_tensor(st[:, ds(D, D)], pc,
                                       r_col[:, ds(ct, 1)], ctx_f[:, ct, :],
                                       op0=Mult, op1=Mult)
        nc.vector.scalar_tensor_tensor(st[:, ds(2 * D, D)], pq,
                                       r_col[:, ds(ct, 1)], ctx_f[:, ct, :],
                                       op0=Mult, op1=Mult)
        nc.gpsimd.dma_start(out_r3[:, ct, ds(2 * D, 2 * D)], st[:, ds(D, 2 * D)])


def _build_bass(loop_n=1):
    import concourse.bass as bass  # noqa: F401
    import concourse.mybir as mybir
    import concourse.tile as tile
    from concourse import bacc

    f32 = mybir.dt.float32

    f32r = mybir.dt.float32r
    nc = bacc.Bacc("TRN2", debug=False, num_devices=N_CORES)
    q_d = nc.dram_tensor("query", [Q, D], f32r, kind="ExternalInput")
    c_d = nc.dram_tensor("context", [C, D], f32r, kind="ExternalInput")
    qw_d = nc.dram_tensor("query_weights", [D, 1], f32r, kind="ExternalInput")
    cw_d = nc.dram_tensor("context_weights", [D, 1], f32r, kind="ExternalInput")
    w_d = nc.dram_tensor("dot_weights", [D, D], f32r, kind="ExternalInput")
    out_d = nc.dram_tensor("out", [C, 4 * D], f32, kind="ExternalOutput")

    aps = (
        q_d.ap().rearrange("(t p) d -> p t d", p=P),
        c_d.ap().rearrange("(t p) d -> p t d", p=P),
        w_d.ap().rearrange("(t p) e -> p t e", p=P),
        cw_d.ap().rearrange("(t p) o -> p t o", p=P),
        qw_d.ap().rearrange("(t p) o -> p t o", p=P),
        out_d.ap().rearrange("(t p) f -> p t f", p=P),
    )

    with tile.TileContext(nc) as tc:
        with (
            tc.tile_pool(name="const", bufs=1) as constp,
            tc.tile_pool(name="stats", bufs=1) as statp,
            tc.tile_pool(name="ctxp", bufs=1) as ctxp,
            tc.tile_pool(name="cbf", bufs=1) as cbfp,
            tc.tile_pool(name="qfam", bufs=1) as qfam,
            tc.tile_pool(name="qbf", bufs=1) as qbfp,
            tc.tile_pool(name="rot8", bufs=2) as rot8,
            tc.tile_pool(name="gpool", bufs=2) as gpool,
            tc.tile_pool(name="fpool", bufs=1) as fpool,
            tc.tile_pool(name="simp", bufs=1) as simp,
            tc.tile_pool(name="cTp", bufs=1) as cTp,
            tc.tile_pool(name="ETp", bufs=1) as ETp,
            tc.tile_pool(name="stage", bufs=3) as stagep,
            tc.tile_pool(name="ps_mm", bufs=4, space="PSUM") as ps_mm,
            tc.tile_pool(name="ps_tr", bufs=3, space="PSUM") as ps_tr,
            tc.tile_pool(name="ps_st", bufs=1, space="PSUM") as ps_st,
        ):
            pools = (statp, ctxp, cbfp, qfam, qbfp, rot8, gpool, fpool,
                     simp, cTp, ETp, stagep, ps_mm, ps_tr, ps_st)
            consts = _emit_consts(nc, constp)
            if loop_n > 1:
                # unroll several bodies per loop iteration: the For_i
                # all-engine barrier is expensive on this runtime, so
                # amortize it while keeping exactly loop_n body runs
                k = 16 if loop_n % 16 == 0 else (
                    8 if loop_n % 8 == 0 else (4 if loop_n % 4 == 0 else 1))
                with tc.For_i(0, loop_n // k, 1):
                    for _ in range(k):
                        _emit_body(nc, tc, pools, aps, consts)
            else:
                _emit_body(nc, tc, pools, aps, consts)
    nc.compile()
    return nc


def get_nc(loop_n=1):
    if loop_n not in _NC_CACHE:
        _NC_CACHE[loop_n] = _build_bass(loop_n)
    return _NC_CACHE[loop_n]


def kernel(query, context, query_weights, context_weights, dot_weights,
           mask=None):
    from concourse.bass_utils import run_bass_kernel_spmd

    query = np.ascontiguousarray(np.asarray(query, dtype=np.float32))
    context = np.ascontiguousarray(np.asarray(context, dtype=np.float32))
    query_weights = np.ascontiguousarray(np.asarray(query_weights, dtype=np.float32))
    context_weights = np.ascontiguousarray(np.asarray(context_weights, dtype=np.float32))
    dot_weights = np.ascontiguousarray(np.asarray(dot_weights, dtype=np.float32))
    # mask is all-True per the problem spec; NEG_INF * (~mask) == 0, so it
    # drops out of the computation entirely.

    nc = get_nc()
    in_maps = [
        {
            "query": query[b],
            "context": context[b],
            "query_weights": query_weights,
            "context_weights": context_weights,
            "dot_weights": dot_weights,
        }
        for b in range(B)
    ]
    res = run_bass_kernel_spmd(nc, in_maps, core_ids=list(range(N_CORES)))
    out = np.stack([res.results[b]["out"] for b in range(B)], axis=0)
    return np.ascontiguousarray(out.astype(np.float32))


if __name__ == "__main__":
    rng = np.random.default_rng(0)
    inputs = {
        "query": rng.standard_normal((B, Q, D), dtype=np.float32),
        "context": rng.standard_normal((B, C, D), dtype=np.float32),
        "query_weights": rng.standard_normal((D, 1), dtype=np.float32) * 0.05,
        "context_weights": rng.standard_normal((D, 1), dtype=np.float32) * 0.05,
        "dot_weights": rng.standard_normal((D, D), dtype=np.float32) * 0.05,
        "mask": np.ones((B, C, Q), dtype=bool),
    }
    out = kernel(**inputs)
    print("out", out.shape, out.dtype)
